# revision 1
# baseline (speedup 1.0000x reference)
"""AttnBlock (GroupNorm + single-head 1x1-conv attention + residual) on 8
Trainium2 NeuronCores.

Sharding: data-parallel over batch (4) x sequence-parallel over query tokens
(2 halves of 4096). Each core receives its batch element with the spatial
columns rotated so that its 2048 query tokens are always columns 0:2047 —
attention is invariant to key order, so one shared NEFF serves all cores.

Compute dtype: fp16 on the PE (full-rate), fp32 PSUM accumulation, fp32
softmax denominators and GroupNorm statistics.
"""

import numpy as np

P = 128
C = 512
KC = C // P          # 4 channel chunks of 128
N = 4096             # tokens (64*64)
NH = N // 2          # query tokens per core
G = 32               # groupnorm groups
GS = C // G          # 16 channels per group
EPS = 1e-6
N_CORES = 8

_CACHE = {}


def _apply_walrus_workarounds():
    """The walrus build in this container rejects any instruction carrying
    more than one semaphore wait ("Too many sync wait commands"). Split extra
    waits onto same-engine single-wait NOPs committed just before, and split
    the final TileContext drain the same way."""
    import concourse.tile as tile
    from concourse import mybir

    if getattr(tile.TileContext, "_walrus_wait_split", False):
        return

    _orig_commit = tile.TileContext._commit_instruction

    def _split_waits_commit(self, inst, lazy_reg_writes=True):
        si = inst.sync_info
        if si is not None and si.on_wait and len(si.on_wait) > 1 \
                and inst.engine != mybir.EngineType.Unassigned:
            waits = list(si.on_wait)
            si.on_wait = waits[-1:]
            for w in waits[:-1]:
                nop = mybir.InstNoOp(
                    name=self.nc.get_next_instruction_name(),
                    engine=inst.engine,
                    sync_info=mybir.SyncInfo(on_wait=[w], on_update=[]),
                    bass_nofuse=True,
                )
                _orig_commit(self, nop, lazy_reg_writes=False)
        return _orig_commit(self, inst, lazy_reg_writes=lazy_reg_writes)

    def _split_drain_and_barrier(self, tick_clock, wait_clock):
        nc = self.nc
        drain_inst = nc.sync.drain()
        wait_clock.add_sem_waits(
            drain_inst.ins, tile.ScopedClock({None: tick_clock.global_clock})
        )
        si = drain_inst.ins.sync_info
        waits = list(si.on_wait) if si is not None else []
        if len(waits) > 1:
            si.on_wait = waits[:1]
            for w in waits[1:]:
                d2 = nc.sync.drain()
                d2.ins.sync_info = mybir.SyncInfo(on_wait=[w], on_update=[])

        import os
        nc.all_engine_barrier()
        assert self.sems is not None
        popped = nc._tile_sem_poison_stack.pop()
        assert popped is self._sem_poison
        if os.environ.get("KERNEL_SKIP_SEM_RESET") != "1":
            nc.clear_and_free_semaphores(list(self.sems.allocated().values()))
            nc.all_engine_barrier()

    tile.TileContext._commit_instruction = _split_waits_commit
    tile.TileContext._drain_and_barrier = _split_drain_and_barrier
    tile.TileContext._walrus_wait_split = True


def _build():
    """Trace the Bass/Tile program once; returns the Bass module."""
    import concourse.bass as bass
    import concourse.tile as tile
    from concourse import mybir

    _apply_walrus_workarounds()

    DT = mybir.dt.float16
    F32 = mybir.dt.float32

    nc = bass.Bass("TRN2", target_bir_lowering=False, debug=False, num_devices=1)

    xr = nc.dram_tensor("xr", [C, N], DT, kind="ExternalInput").ap()
    wq = nc.dram_tensor("wq", [C, C], DT, kind="ExternalInput").ap()
    wk = nc.dram_tensor("wk", [C, C], DT, kind="ExternalInput").ap()
    wv = nc.dram_tensor("wv", [C, C], DT, kind="ExternalInput").ap()
    wo = nc.dram_tensor("wo", [C, C], DT, kind="ExternalInput").ap()
    # packed per-channel vectors: [bq, bk, bo, gamma, beta]
    bvec = nc.dram_tensor("bvec", [5, C], F32, kind="ExternalInput").ap()
    gavg = nc.dram_tensor("gavg", [P, P], F32, kind="ExternalInput").ap()
    ident = nc.dram_tensor("ident", [P, P], DT, kind="ExternalInput").ap()
    y = nc.dram_tensor("y", [C, NH], F32, kind="ExternalOutput").ap()

    xr_t = xr.rearrange("(kc p) n -> kc p n", p=P)     # [4, 128, 4096]
    y_t = y.rearrange("(oc p) n -> oc p n", p=P)       # [4, 128, 2048]

    IB = NH // P        # 16 query blocks per core
    JQ = N // 512       # 8 key chunks of 512
    NHQ = NH // 512     # 4 query-token chunks of 512

    with tile.TileContext(nc) as tc:
        import contextlib
        ctx = contextlib.ExitStack()
        with ctx:
            consts = ctx.enter_context(tc.tile_pool(name="consts", bufs=1))
            big = ctx.enter_context(tc.tile_pool(name="big", bufs=1))
            small = ctx.enter_context(tc.tile_pool(name="small", bufs=4))
            epool = ctx.enter_context(tc.tile_pool(name="epool", bufs=3))
            rpool = ctx.enter_context(tc.tile_pool(name="rpool", bufs=3))
            ps = ctx.enter_context(tc.tile_pool(name="ps", bufs=8, space="PSUM"))

            # ---- phase 1: GroupNorm -> hn (fp16) --------------------------
            # x stays fully resident in SBUF (also serves the phase-4
            # residual). x DMAs are traced first so they win the early HBM
            # bandwidth; consts ride the gpsimd SWDGE queue instead.
            hn = big.tile([P, KC, N], DT, tag="ho")
            x_full = big.tile([P, KC, N], DT, tag="xf")
            bv_sb = None
            for kc in range(KC):
                x_c = x_full[:, kc, :]
                nc.sync.dma_start(x_c[:], xr_t[kc])
                # raw per-partition sum (DVE, 2x on fp16) and sum of squares
                # (ScalarE Square with fused accumulator; hn[:, kc] is
                # throwaway scratch, overwritten by the real hn below). The
                # 1/(GS*N) normalization is folded into the host gavg matrix.
                mv2 = small.tile([P, 2], F32, tag="mv2")
                nc.vector.tensor_reduce(
                    mv2[:, 0:1], x_c[:], mybir.AxisListType.X,
                    mybir.AluOpType.add)
                nc.scalar.activation(
                    hn[:, kc, :], x_c[:], mybir.ActivationFunctionType.Square,
                    accum_out=mv2[:, 1:2])
                if bv_sb is None:
                    bv_sb = consts.tile([P, 5, KC], F32, tag="bvec")
                    nc.gpsimd.dma_start(
                        bv_sb[:], bvec.rearrange("v (kc p) -> p v kc", p=P))
                    b_sb = {n: bv_sb[:, vi, :] for vi, n in
                            enumerate(("bq", "bk", "bo", "gam", "bet"))}
                    gavg_sb = consts.tile([P, P], F32, tag="gavg")
                    nc.gpsimd.dma_start(gavg_sb[:], gavg)
                    ident_sb = consts.tile([P, P], DT, tag="ident")
                    nc.gpsimd.dma_start(ident_sb[:], ident)
                    eps_sb = consts.tile([P, 1], F32, tag="eps")
                    nc.vector.memset(eps_sb[:], EPS)
                # group-average (and broadcast back to partitions) via PE
                g_ps = ps.tile([P, 2], F32, tag="mm", name=f"gn{kc}")
                nc.tensor.matmul(g_ps[:], gavg_sb[:], mv2[:], start=True, stop=True)

                # var_g = E2_g - mean_g^2 ; rstd = 1/sqrt(var_g + eps)
                g_sb = small.tile([P, 2], F32, tag="gsb")
                nc.vector.tensor_copy(g_sb[:], g_ps[:])
                var_t = small.tile([P, 1], F32, tag="var")
                nc.gpsimd.tensor_tensor(
                    var_t[:], g_sb[:, 0:1], g_sb[:, 0:1], mybir.AluOpType.mult)
                nc.gpsimd.tensor_tensor(
                    var_t[:], g_sb[:, 1:2], var_t[:], mybir.AluOpType.subtract)
                sq = small.tile([P, 1], F32, tag="sq")
                nc.scalar.activation(
                    sq[:], var_t[:], mybir.ActivationFunctionType.Sqrt,
                    bias=eps_sb[:], scale=1.0)
                rstd = small.tile([P, 1], F32, tag="rstd")
                nc.vector.reciprocal(rstd[:], sq[:])

                # scale = rstd * gamma ; shift = beta - mean_g * scale
                scl = small.tile([P, 1], F32, tag="scl")
                nc.gpsimd.tensor_tensor(
                    scl[:], rstd[:], b_sb["gam"][:, kc:kc + 1], mybir.AluOpType.mult)
                sh = small.tile([P, 1], F32, tag="sh")
                nc.gpsimd.tensor_tensor(
                    sh[:], g_sb[:, 0:1], scl[:], mybir.AluOpType.mult)
                nc.gpsimd.tensor_tensor(
                    sh[:], b_sb["bet"][:, kc:kc + 1], sh[:], mybir.AluOpType.subtract)

                nc.vector.tensor_scalar(
                    out=hn[:, kc, :], in0=x_c[:], scalar1=scl[:], scalar2=sh[:],
                    op0=mybir.AluOpType.mult, op1=mybir.AluOpType.add)

            # HAM warm-up: dummy matmuls gated on the second-to-last hn
            # chunk keep the PE busy through the idle tail of phase 1 so
            # phase 2 starts at 2.4 GHz instead of the cold 1.2 GHz.
            warm_ps = ps.tile([P, 512], F32, tag="mm", name="warm")
            for wi in range(8):
                nc.tensor.matmul(warm_ps[:], ident_sb[:], hn[:, 2, :512],
                                 start=(wi == 0), stop=(wi == 7))

            # weights (first needed by phase 2)
            w_sb = {}
            for name, ap in (("wk", wk), ("wq", wq), ("wv", wv), ("wo", wo)):
                t = consts.tile([P, KC, C], DT, tag=f"w_{name}")
                nc.gpsimd.dma_start(t[:], ap.rearrange("(kc p) o -> p kc o", p=P))
                w_sb[name] = t

            # ---- phase 2: projections ------------------------------------
            k_sb = big.tile([P, KC, N], DT, tag="k")
            q_sb = big.tile([P, KC, NH], DT, tag="q")
            vt_sb = big.tile([P, N // P, C], DT, tag="vt")

            for oc in range(KC):
                for nt in range(JQ):
                    pp = ps.tile([P, 512], F32, tag="mm")
                    for kc in range(KC):
                        nc.tensor.matmul(
                            pp[:], w_sb["wk"][:, kc, oc * P:(oc + 1) * P],
                            hn[:, kc, nt * 512:(nt + 1) * 512],
                            start=(kc == 0), stop=(kc == KC - 1))
                    nc.scalar.activation(
                        k_sb[:, oc, nt * 512:(nt + 1) * 512], pp[:],
                        mybir.ActivationFunctionType.Identity,
                        bias=b_sb["bk"][:, oc:oc + 1], scale=1.0)
            for oc in range(KC):
                for nt in range(NHQ):
                    pp = ps.tile([P, 512], F32, tag="mm")
                    for kc in range(KC):
                        nc.tensor.matmul(
                            pp[:], w_sb["wq"][:, kc, oc * P:(oc + 1) * P],
                            hn[:, kc, nt * 512:(nt + 1) * 512],
                            start=(kc == 0), stop=(kc == KC - 1))
                    nc.scalar.activation(
                        q_sb[:, oc, nt * 512:(nt + 1) * 512], pp[:],
                        mybir.ActivationFunctionType.Identity,
                        bias=b_sb["bq"][:, oc:oc + 1], scale=1.0)
            for jc in range(N // P):
                pp = ps.tile([P, 512], F32, tag="mm")
                for kc in range(KC):
                    nc.tensor.matmul(
                        pp[:], hn[:, kc, jc * P:(jc + 1) * P], w_sb["wv"][:, kc, :],
                        start=(kc == 0), stop=(kc == KC - 1))
                nc.vector.tensor_copy(vt_sb[:, jc, :], pp[:])

            # ---- phase 3: attention, 16 query blocks ---------------------
            # Flat software pipeline over global key chunks u = ib*JQ + jq:
            #   iter t:  S-matmuls + exp of chunk t
            #            transposes + AT copy + O^T matmuls of chunk t-1
            #            epilogue of block (t-2)//JQ when t-2 ends a block
            # so the PE never sits on the exp (ACT) latency of its own chunk.
            o_sb = big.tile([P, KC, NH], DT, tag="ho", name="o_sb")
            TOT = IB * JQ
            e_hold = {}
            ssum_hold = {}
            ot_hold = {}

            def stage_s(u):
                ib, jq = divmod(u, JQ)
                if jq == 0:
                    ssum_hold[ib] = small.tile([P, JQ], F32, tag="ssum", name=f"ssum{ib}")
                s_ps = ps.tile([P, 512], F32, tag="mm")
                for kc in range(KC):
                    nc.tensor.matmul(
                        s_ps[:], q_sb[:, kc, ib * P:(ib + 1) * P],
                        k_sb[:, kc, jq * 512:(jq + 1) * 512],
                        start=(kc == 0), stop=(kc == KC - 1))
                e_sb = epool.tile([P, 512], DT, tag="e")
                nc.scalar.activation(
                    e_sb[:], s_ps[:], mybir.ActivationFunctionType.Exp,
                    accum_out=ssum_hold[ib][:, jq:jq + 1])
                e_hold[u] = e_sb

            at_hold = {}

            def stage_t(u):
                e_sb = e_hold.pop(u)
                t_ps = ps.tile([P, 512], DT, tag="mm")
                for jj in range(4):
                    nc.tensor.transpose(
                        t_ps[:, jj * P:(jj + 1) * P],
                        e_sb[:, jj * P:(jj + 1) * P], ident_sb[:])
                at_sb = epool.tile([P, 4, P], DT, tag="at")
                nc.vector.tensor_copy(
                    at_sb[:], t_ps.rearrange("p (a b) -> p a b", b=P))
                at_hold[u] = at_sb

            def stage_ot(u):
                ib, jq = divmod(u, JQ)
                if jq == 0:
                    ot_hold[ib] = ps.tile([P, C], F32, tag="mm", name=f"ot{ib}")
                ot_ps = ot_hold[ib]
                at_sb = at_hold.pop(u)
                for jj in range(4):
                    nc.tensor.matmul(
                        ot_ps[:], at_sb[:, jj, :], vt_sb[:, jq * 4 + jj, :],
                        start=(jq == 0 and jj == 0),
                        stop=(jq == JQ - 1 and jj == 3))

            def stage_epi(ib):
                ssum = ssum_hold.pop(ib)
                ot_ps = ot_hold.pop(ib)
                ssum_r = small.tile([P, 1], F32, tag="ssum_r")
                nc.vector.tensor_reduce(
                    ssum_r[:], ssum[:], mybir.AxisListType.X, mybir.AluOpType.add)
                recip = small.tile([P, 1], F32, tag="recip")
                nc.vector.reciprocal(recip[:], ssum_r[:])

                ot_sb = epool.tile([P, C], DT, tag="ot")
                nc.scalar.activation(
                    ot_sb[:], ot_ps[:], mybir.ActivationFunctionType.Copy,
                    scale=recip[:])
                to_ps = ps.tile([P, C], DT, tag="mm")
                for cb in range(KC):
                    nc.tensor.transpose(
                        to_ps[:, cb * P:(cb + 1) * P],
                        ot_sb[:, cb * P:(cb + 1) * P], ident_sb[:])
                nc.vector.tensor_copy(
                    o_sb[:, :, ib * P:(ib + 1) * P],
                    to_ps.rearrange("p (a b) -> p a b", b=P))

            # ---- phase 4 (interleaved): output projection + residual -----
            def stage_out_oc(nt, oc):
                pp = ps.tile([P, 512], F32, tag="mm")
                for kc in range(KC):
                    nc.tensor.matmul(
                        pp[:], w_sb["wo"][:, kc, oc * P:(oc + 1) * P],
                        o_sb[:, kc, nt * 512:(nt + 1) * 512],
                        start=(kc == 0), stop=(kc == KC - 1))
                r_sb = rpool.tile([P, 512], F32, tag="r")
                nc.scalar.activation(
                    r_sb[:], pp[:], mybir.ActivationFunctionType.Identity,
                    bias=b_sb["bo"][:, oc:oc + 1], scale=1.0)
                nc.vector.tensor_tensor(
                    r_sb[:], r_sb[:], x_full[:, oc, nt * 512:(nt + 1) * 512],
                    mybir.AluOpType.add)
                nc.sync.dma_start(y_t[oc][:, nt * 512:(nt + 1) * 512], r_sb[:])

            for t in range(TOT + 8):
                if t < TOT:
                    stage_s(t)
                if 1 <= t <= TOT:
                    stage_t(t - 1)
                if 2 <= t <= TOT + 1:
                    stage_ot(t - 2)
                if t >= 4 and (t - 4) % JQ == JQ - 1 and (t - 4) // JQ < IB:
                    stage_epi((t - 4) // JQ)
                # wo-projection for token slice nt once blocks 4nt..4nt+3
                # are epilogued; one oc group per iteration so the final
                # burst isn't exposed at the kernel tail.
                for oc in range(KC):
                    tt = t - 5 - oc
                    if tt >= 0 and tt % (4 * JQ) == 4 * JQ - 1 \
                            and tt // (4 * JQ) < NHQ:
                        stage_out_oc(tt // (4 * JQ), oc)

    return nc


def _prep_in_maps(inputs):
    x = np.asarray(inputs["x"], np.float32).reshape(4, C, N)
    s = np.float32(C ** -0.5)
    wq = np.asarray(inputs["wq"], np.float32)
    wk = np.asarray(inputs["wk"], np.float32)
    wv = np.asarray(inputs["wv"], np.float32)
    wo = np.asarray(inputs["wo"], np.float32)
    bvec = np.stack([
        np.asarray(inputs["bq"], np.float32) * s,
        np.asarray(inputs["bk"], np.float32),
        wo @ np.asarray(inputs["bv"], np.float32)
        + np.asarray(inputs["bo"], np.float32),
        np.asarray(inputs["gamma"], np.float32),
        np.asarray(inputs["beta"], np.float32),
    ]).astype(np.float32)
    shared = {
        "wq": np.ascontiguousarray((wq * s).T).astype(np.float16),
        "wk": np.ascontiguousarray(wk.T).astype(np.float16),
        "wv": np.ascontiguousarray(wv.T).astype(np.float16),
        "wo": np.ascontiguousarray(wo.T).astype(np.float16),
        "bvec": bvec,
        "gavg": (np.kron(np.eye(P // GS, dtype=np.float32),
                         np.ones((GS, GS), np.float32)) / (GS * N)),
        "ident": np.eye(P, dtype=np.float16),
    }
    in_maps = []
    for core in range(N_CORES):
        b, half = divmod(core, 2)
        xb = x[b]
        if half == 1:
            xrot = np.ascontiguousarray(
                np.concatenate([xb[:, NH:], xb[:, :NH]], axis=1))
        else:
            xrot = np.ascontiguousarray(xb)
        in_maps.append({"xr": xrot.astype(np.float16), **shared})
    return in_maps


def kernel_run(inputs, trace=False, trace_cores=None):
    """Run on all 8 cores; returns (full_output, BassKernelResults)."""
    from concourse.bass_utils import run_bass_kernel_spmd

    if "nc" not in _CACHE:
        _CACHE["nc"] = _build()
    nc = _CACHE["nc"]
    in_maps = _prep_in_maps(inputs)
    res = run_bass_kernel_spmd(
        nc, in_maps, core_ids=list(range(N_CORES)), trace=trace,
        trace_cores=trace_cores)
    out = np.empty((4, C, N), np.float32)
    for core in range(N_CORES):
        b, half = divmod(core, 2)
        out[b][:, half * NH:(half + 1) * NH] = res.results[core]["y"]
    return out.reshape(4, C, 64, 64), res


def kernel(**inputs):
    out, _ = kernel_run(inputs, trace=False)
    return out



# revision 5
# speedup vs baseline: 1.6057x; 1.6057x over previous
"""AttnBlock (GroupNorm + single-head 1x1-conv attention + residual) on 8
Trainium2 NeuronCores.

Sharding: data-parallel over batch (4) x sequence-parallel over query tokens
(2 halves of 4096). Each core receives its batch element with the spatial
columns rotated so that its 2048 query tokens are always columns 0:2047 —
attention is invariant to key order, so one shared NEFF serves all cores.

Math/layout tricks vs the fp16 baseline:
  * All big matmuls run in fp8(e4m3) DoubleRow mode (2x PE throughput):
    weights, hn, q, k, v, and the attention weights are fp8.
  * Scores are computed TRANSPOSED (S^T[key, query]) so exp() output lands
    directly in the [key, query] layout the attn@V matmul needs as lhsT —
    no PE transposes and no DVE copies of the 16M-element score matrix.
  * Softmax denominators ride on piggy-backed DoubleRow matmuls that reuse
    the attention tile as stationary weights against a constant ones-rhs.
  * The key bias bk drops exactly (adds a per-query constant to scores ->
    cancels in softmax); wo is folded into wv on the host (wv' = wo @ wv),
    eliminating the entire output-projection phase; wo@bv + bo rides the
    residual add (softmax weights sum to one).
  * Weights/activations are pre-scaled by 16 so fp8 values sit in the
    normal-number range; the exp() activation folds the compensating
    1/256 and the C^-0.5 softmax scale into its scale operand.
"""

import numpy as np

P = 128
C = 512
KC = C // P          # 4 channel chunks of 128
N = 4096             # tokens (64*64)
NH = N // 2          # query tokens per core
G = 32               # groupnorm groups
GS = C // G          # 16 channels per group
EPS = 1e-6
N_CORES = 8

SCW = np.float32(16.0)       # fp8 pre-scale on weights/activations
MSH = 3.0                    # exp shift: exp(s - MSH), cancels in softmax
ESC = float(C ** -0.5 / (SCW * SCW))  # exp scale on raw fp8 score psum

NKB = N // P         # 32 key blocks of 128
NQC = NH // 512      # 4 query chunks of 512

_CACHE = {}


def _apply_walrus_workarounds():
    """The walrus build in this container rejects any instruction carrying
    more than one semaphore wait ("Too many sync wait commands"). Split extra
    waits onto same-engine single-wait NOPs committed just before, and split
    the final TileContext drain the same way."""
    import concourse.tile as tile
    from concourse import mybir

    if getattr(tile.TileContext, "_walrus_wait_split", False):
        return

    _orig_commit = tile.TileContext._commit_instruction

    def _split_waits_commit(self, inst, lazy_reg_writes=True):
        si = inst.sync_info
        if si is not None and si.on_wait and len(si.on_wait) > 1 \
                and inst.engine != mybir.EngineType.Unassigned:
            waits = list(si.on_wait)
            si.on_wait = waits[-1:]
            for w in waits[:-1]:
                nop = mybir.InstNoOp(
                    name=self.nc.get_next_instruction_name(),
                    engine=inst.engine,
                    sync_info=mybir.SyncInfo(on_wait=[w], on_update=[]),
                    bass_nofuse=True,
                )
                _orig_commit(self, nop, lazy_reg_writes=False)
        return _orig_commit(self, inst, lazy_reg_writes=lazy_reg_writes)

    def _split_drain_and_barrier(self, tick_clock, wait_clock):
        nc = self.nc
        drain_inst = nc.sync.drain()
        wait_clock.add_sem_waits(
            drain_inst.ins, tile.ScopedClock({None: tick_clock.global_clock})
        )
        si = drain_inst.ins.sync_info
        waits = list(si.on_wait) if si is not None else []
        if len(waits) > 1:
            si.on_wait = waits[:1]
            for w in waits[1:]:
                d2 = nc.sync.drain()
                d2.ins.sync_info = mybir.SyncInfo(on_wait=[w], on_update=[])

        import os
        nc.all_engine_barrier()
        assert self.sems is not None
        popped = nc._tile_sem_poison_stack.pop()
        assert popped is self._sem_poison
        if os.environ.get("KERNEL_SKIP_SEM_RESET") != "1":
            nc.clear_and_free_semaphores(list(self.sems.allocated().values()))
            nc.all_engine_barrier()

    tile.TileContext._commit_instruction = _split_waits_commit
    tile.TileContext._drain_and_barrier = _split_drain_and_barrier
    tile.TileContext._walrus_wait_split = True


def _build():
    """Trace the Bass/Tile program once; returns the Bass module."""
    import concourse.bass as bass
    import concourse.tile as tile
    from concourse import mybir

    _apply_walrus_workarounds()

    DT = mybir.dt.float16
    F8 = mybir.dt.float8e4
    F32 = mybir.dt.float32
    DR = mybir.MatmulPerfMode.DoubleRow

    nc = bass.Bass("TRN2", target_bir_lowering=False, debug=False, num_devices=1)

    xr = nc.dram_tensor("xr", [C, N], DT, kind="ExternalInput").ap()
    wq = nc.dram_tensor("wq", [C, C], F8, kind="ExternalInput").ap()
    wk = nc.dram_tensor("wk", [C, C], F8, kind="ExternalInput").ap()
    wv = nc.dram_tensor("wv", [C, C], F8, kind="ExternalInput").ap()
    # packed per-channel vectors: [16*bq, gamma, beta, wo@bv+bo]
    bvec = nc.dram_tensor("bvec", [4, C], F32, kind="ExternalInput").ap()
    gavg = nc.dram_tensor("gavg", [P, P], F32, kind="ExternalInput").ap()
    ident = nc.dram_tensor("ident", [P, P], DT, kind="ExternalInput").ap()
    y = nc.dram_tensor("y", [C, NH], F32, kind="ExternalOutput").ap()

    xr_t = xr.rearrange("(kc p) n -> kc p n", p=P)     # [4, 128, 4096]
    y_r = y.rearrange("(oc p) n -> p oc n", p=P)       # [128, 4, 2048]

    with tile.TileContext(nc) as tc:
        import contextlib
        ctx = contextlib.ExitStack()
        with ctx:
            consts = ctx.enter_context(tc.tile_pool(name="consts", bufs=1))
            big = ctx.enter_context(tc.tile_pool(name="big", bufs=1))
            small = ctx.enter_context(tc.tile_pool(name="small", bufs=4))
            epool = ctx.enter_context(tc.tile_pool(name="epool", bufs=4))
            rpool = ctx.enter_context(tc.tile_pool(name="rpool", bufs=3))
            ps = ctx.enter_context(tc.tile_pool(name="ps", bufs=8, space="PSUM"))

            # ---- consts (gpsimd SWDGE queue; ident first for PE warmup) ---
            ident_sb = consts.tile([P, P], DT, tag="ident")
            nc.gpsimd.dma_start(ident_sb[:], ident)
            warm_sb = consts.tile([P, 512], DT, tag="warm")
            nc.vector.memset(warm_sb[:], 0.5)
            ones8 = consts.tile([P, 2, P], F8, tag="ones")
            nc.vector.memset(ones8[:], float(SCW))
            expb = consts.tile([P, 1], F32, tag="expb")
            nc.vector.memset(expb[:], -MSH)
            eps_sb = consts.tile([P, 1], F32, tag="eps")
            nc.vector.memset(eps_sb[:], EPS)

            # PE clock warm-up: accumulation chain gated only on ident/warm
            # keeps the PE streaming from ~t=2us so HAM lifts the clock to
            # 2.4GHz while GroupNorm (DVE/ACT-bound) is still running.
            warm_ps = ps.tile([P, 512], F32, tag="mm", name="warm")
            NWARM = 44
            for wi in range(NWARM):
                nc.tensor.matmul(warm_ps[:], ident_sb[:], warm_sb[:],
                                 start=(wi == 0), stop=(wi == NWARM - 1))

            bv_sb = consts.tile([P, 4, KC], F32, tag="bvec")
            nc.gpsimd.dma_start(
                bv_sb[:], bvec.rearrange("v (kc p) -> p v kc", p=P))
            b_sb = {n: bv_sb[:, vi, :] for vi, n in
                    enumerate(("bq", "gam", "bet", "bo"))}
            gavg_sb = consts.tile([P, P], F32, tag="gavg")
            nc.gpsimd.dma_start(gavg_sb[:], gavg)

            # ---- phase 1: GroupNorm -> hn8 (fp8) --------------------------
            # x stays fully resident in SBUF (also serves the residual).
            hn8 = big.tile([P, KC, N], F8, tag="hn")
            x_full = big.tile([P, KC, N], DT, tag="xf")
            for kc in range(KC):
                x_c = x_full[:, kc, :]
                nc.sync.dma_start(x_c[:], xr_t[kc])
                # raw per-partition sum (DVE) and sum of squares (ScalarE
                # Square with fused accumulator; hn8[:, kc] is throwaway
                # scratch). The 1/(GS*N) normalization is folded into gavg.
                mv2 = small.tile([P, 2], F32, tag="mv2")
                nc.vector.tensor_reduce(
                    mv2[:, 0:1], x_c[:], mybir.AxisListType.X,
                    mybir.AluOpType.add)
                nc.scalar.activation(
                    hn8[:, kc, :], x_c[:], mybir.ActivationFunctionType.Square,
                    accum_out=mv2[:, 1:2])
                # group-average (and broadcast back to partitions) via PE
                g_ps = ps.tile([P, 2], F32, tag="mm", name=f"gn{kc}")
                nc.tensor.matmul(g_ps[:], gavg_sb[:], mv2[:], start=True, stop=True)

                # var_g = E2_g - mean_g^2 ; rstd = 1/sqrt(var_g + eps)
                g_sb = small.tile([P, 2], F32, tag="gsb")
                nc.vector.tensor_copy(g_sb[:], g_ps[:])
                var_t = small.tile([P, 1], F32, tag="var")
                nc.gpsimd.tensor_tensor(
                    var_t[:], g_sb[:, 0:1], g_sb[:, 0:1], mybir.AluOpType.mult)
                nc.gpsimd.tensor_tensor(
                    var_t[:], g_sb[:, 1:2], var_t[:], mybir.AluOpType.subtract)
                sq = small.tile([P, 1], F32, tag="sq")
                nc.scalar.activation(
                    sq[:], var_t[:], mybir.ActivationFunctionType.Sqrt,
                    bias=eps_sb[:], scale=1.0)
                rstd = small.tile([P, 1], F32, tag="rstd")
                nc.vector.reciprocal(rstd[:], sq[:])

                # scale = rstd * gamma ; shift = beta - mean_g * scale
                scl = small.tile([P, 1], F32, tag="scl")
                nc.gpsimd.tensor_tensor(
                    scl[:], rstd[:], b_sb["gam"][:, kc:kc + 1], mybir.AluOpType.mult)
                sh = small.tile([P, 1], F32, tag="sh")
                nc.gpsimd.tensor_tensor(
                    sh[:], g_sb[:, 0:1], scl[:], mybir.AluOpType.mult)
                nc.gpsimd.tensor_tensor(
                    sh[:], b_sb["bet"][:, kc:kc + 1], sh[:], mybir.AluOpType.subtract)

                nc.vector.tensor_scalar(
                    out=hn8[:, kc, :], in0=x_c[:], scalar1=scl[:], scalar2=sh[:],
                    op0=mybir.AluOpType.mult, op1=mybir.AluOpType.add)

            # weights (first needed by phase 2)
            w_sb = {}
            for name, ap in (("wk", wk), ("wq", wq), ("wv", wv)):
                t = consts.tile([P, KC, C], F8, tag=f"w_{name}")
                nc.gpsimd.dma_start(t[:], ap.rearrange("(kc p) o -> p kc o", p=P))
                w_sb[name] = t

            # ---- phase 2: projections (fp8 DoubleRow) --------------------
            k8 = big.tile([P, KC, N], F8, tag="k")
            q8 = big.tile([P, KC, NH], F8, tag="q")
            vt8 = big.tile([P, NKB, C], F8, tag="vt")

            for nt in range(N // 512):
                for oc in range(KC):
                    pp = ps.tile([P, 512], F32, tag="mm")
                    for kcp in range(2):
                        nc.tensor.matmul(
                            pp[:],
                            w_sb["wk"][:, 2 * kcp:2 * kcp + 2, oc * P:(oc + 1) * P],
                            hn8[:, 2 * kcp:2 * kcp + 2, nt * 512:(nt + 1) * 512],
                            start=(kcp == 0), stop=(kcp == 1), perf_mode=DR)
                    # bk cancels in softmax: pure cast epilogue
                    nc.vector.tensor_copy(
                        k8[:, oc, nt * 512:(nt + 1) * 512], pp[:])
            for nt in range(NQC):
                for oc in range(KC):
                    pp = ps.tile([P, 512], F32, tag="mm")
                    for kcp in range(2):
                        nc.tensor.matmul(
                            pp[:],
                            w_sb["wq"][:, 2 * kcp:2 * kcp + 2, oc * P:(oc + 1) * P],
                            hn8[:, 2 * kcp:2 * kcp + 2, nt * 512:(nt + 1) * 512],
                            start=(kcp == 0), stop=(kcp == 1), perf_mode=DR)
                    nc.vector.tensor_scalar_add(
                        q8[:, oc, nt * 512:(nt + 1) * 512], pp[:],
                        b_sb["bq"][:, oc:oc + 1])
            for jc in range(NKB):
                pp = ps.tile([P, 512], F32, tag="mm")
                for kcp in range(2):
                    nc.tensor.matmul(
                        pp[:],
                        hn8[:, 2 * kcp:2 * kcp + 2, jc * P:(jc + 1) * P],
                        w_sb["wv"][:, 2 * kcp:2 * kcp + 2, :],
                        start=(kcp == 0), stop=(kcp == 1), perf_mode=DR)
                nc.vector.tensor_copy(vt8[:, jc, :], pp[:])

            # ---- phase 3: attention over transposed scores ---------------
            for qc in range(NQC):
                qsl = slice(qc * 512, (qc + 1) * 512)
                den_ps = ps.tile([P, KC, P], F32, tag="mm", name=f"den{qc}")
                ot_ps = [ps.tile([P, C], F32, tag="mm", name=f"ot{qc}_{qb}")
                         for qb in range(4)]
                at_hold = {}

                def stage_s(kb, qc=qc, qsl=qsl, at_hold=at_hold):
                    p = kb // 2
                    if kb % 2 == 0:
                        at_hold[p] = epool.tile([P, 2, 512], F8, tag="at",
                                                name=f"at{qc}_{p}")
                    s_ps = ps.tile([P, 512], F32, tag="mm")
                    for kcp in range(2):
                        nc.tensor.matmul(
                            s_ps[:],
                            k8[:, 2 * kcp:2 * kcp + 2, kb * P:(kb + 1) * P],
                            q8[:, 2 * kcp:2 * kcp + 2, qsl],
                            start=(kcp == 0), stop=(kcp == 1), perf_mode=DR)
                    nc.scalar.activation(
                        at_hold[p][:, kb % 2, :], s_ps[:],
                        mybir.ActivationFunctionType.Exp,
                        bias=expb[:], scale=ESC)

                def stage_ot(p, den_ps=den_ps, ot_ps=ot_ps, at_hold=at_hold):
                    at = at_hold.pop(p)
                    for qb in range(4):
                        nc.tensor.matmul(
                            ot_ps[qb][:], at[:, :, qb * P:(qb + 1) * P],
                            vt8[:, 2 * p:2 * p + 2, :],
                            start=(p == 0), stop=(p == NKB // 2 - 1),
                            perf_mode=DR)
                        nc.tensor.matmul(
                            den_ps[:, qb, :], at[:, :, qb * P:(qb + 1) * P],
                            ones8[:],
                            start=(p == 0), stop=(p == NKB // 2 - 1),
                            perf_mode=DR)

                for kb in range(NKB):
                    stage_s(kb)
                    if kb % 2 == 1 and kb >= 3:
                        stage_ot((kb - 3) // 2)
                stage_ot(NKB // 2 - 1)

                # epilogue: scale by 1/den, transpose back to [c, q], add
                # residual + folded bias, DMA out. Two halves of 256 queries.
                tp = {}
                for qb in range(4):
                    recip = small.tile([P, 1], F32, tag="recip")
                    nc.vector.reciprocal(recip[:], den_ps[:, qb, 0:1])
                    ot_sb = rpool.tile([P, C], DT, tag="ot")
                    nc.scalar.activation(
                        ot_sb[:], ot_ps[qb][:],
                        mybir.ActivationFunctionType.Copy, scale=recip[:])
                    j, jj = divmod(qb, 2)
                    if jj == 0:
                        tp[j] = ps.tile([P, KC, 256], DT, tag="mm",
                                        name=f"tp{qc}_{j}")
                    for cb in range(KC):
                        nc.tensor.transpose(
                            tp[j][:, cb, jj * P:(jj + 1) * P],
                            ot_sb[:, cb * P:(cb + 1) * P], ident_sb[:])
                for j in range(2):
                    r1 = rpool.tile([P, KC, 256], F32, tag="r1")
                    rsl = slice(qc * 512 + j * 256, qc * 512 + (j + 1) * 256)
                    for cb in range(KC):
                        nc.vector.tensor_scalar_add(
                            r1[:, cb, :], tp[j][:, cb, :],
                            b_sb["bo"][:, cb:cb + 1])
                        nc.gpsimd.tensor_tensor(
                            r1[:, cb, :], r1[:, cb, :], x_full[:, cb, rsl],
                            mybir.AluOpType.add)
                    nc.sync.dma_start(y_r[:, :, rsl], r1[:])

    return nc


def _prep_in_maps(inputs):
    import ml_dtypes
    f8 = ml_dtypes.float8_e4m3

    x = np.asarray(inputs["x"], np.float32).reshape(4, C, N)
    wq = np.asarray(inputs["wq"], np.float32)
    wk = np.asarray(inputs["wk"], np.float32)
    wv = np.asarray(inputs["wv"], np.float32)
    wo = np.asarray(inputs["wo"], np.float32)
    wvp = wo @ wv                     # fold output projection into v
    bvec = np.stack([
        np.asarray(inputs["bq"], np.float32) * SCW,
        np.asarray(inputs["gamma"], np.float32),
        np.asarray(inputs["beta"], np.float32),
        wo @ np.asarray(inputs["bv"], np.float32)
        + np.asarray(inputs["bo"], np.float32),
    ]).astype(np.float32)
    shared = {
        "wq": np.ascontiguousarray((wq * SCW).T).astype(f8),
        "wk": np.ascontiguousarray((wk * SCW).T).astype(f8),
        "wv": np.ascontiguousarray((wvp * SCW).T).astype(f8),
        "bvec": bvec,
        "gavg": (np.kron(np.eye(P // GS, dtype=np.float32),
                         np.ones((GS, GS), np.float32)) / (GS * N)),
        "ident": np.eye(P, dtype=np.float16),
    }
    in_maps = []
    for core in range(N_CORES):
        b, half = divmod(core, 2)
        xb = x[b]
        if half == 1:
            xrot = np.ascontiguousarray(
                np.concatenate([xb[:, NH:], xb[:, :NH]], axis=1))
        else:
            xrot = np.ascontiguousarray(xb)
        in_maps.append({"xr": xrot.astype(np.float16), **shared})
    return in_maps


def kernel_run(inputs, trace=False, trace_cores=None):
    """Run on all 8 cores; returns (full_output, BassKernelResults)."""
    from concourse.bass_utils import run_bass_kernel_spmd

    if "nc" not in _CACHE:
        _CACHE["nc"] = _build()
    nc = _CACHE["nc"]
    in_maps = _prep_in_maps(inputs)
    res = run_bass_kernel_spmd(
        nc, in_maps, core_ids=list(range(N_CORES)), trace=trace,
        trace_cores=trace_cores)
    out = np.empty((4, C, N), np.float32)
    for core in range(N_CORES):
        b, half = divmod(core, 2)
        out[b][:, half * NH:(half + 1) * NH] = res.results[core]["y"]
    return out.reshape(4, C, 64, 64), res


def kernel(**inputs):
    out, _ = kernel_run(inputs, trace=False)
    return out


# revision 11
# speedup vs baseline: 1.6743x; 1.0427x over previous
"""AttnBlock (GroupNorm + single-head 1x1-conv attention + residual) on 8
Trainium2 NeuronCores.

Sharding: data-parallel over batch (4) x sequence-parallel over query tokens
(2 halves of 4096). Each core receives its batch element with the spatial
columns rotated so that its 2048 query tokens are always columns 0:2047 —
attention is invariant to key order, so one shared NEFF serves all cores.

Math/layout tricks vs the fp16 baseline:
  * All big matmuls run in fp8(e4m3) DoubleRow mode (2x PE throughput):
    weights, hn, q, k, v, and the attention weights are fp8.
  * Scores are computed TRANSPOSED (S^T[key, query]) so exp() output lands
    directly in the [key, query] layout the attn@V matmul needs as lhsT —
    no PE transposes and no DVE copies of the 16M-element score matrix.
  * Softmax denominators ride on piggy-backed DoubleRow matmuls that reuse
    the attention tile as stationary weights against a constant ones-rhs.
  * The key bias bk drops exactly (adds a per-query constant to scores ->
    cancels in softmax); wo is folded into wv on the host (wv' = wo @ wv),
    eliminating the entire output-projection phase; wo@bv + bo rides the
    residual add (softmax weights sum to one).
  * Weights/activations are pre-scaled by 16 so fp8 values sit in the
    normal-number range; the exp() activation folds the compensating
    1/256 and the C^-0.5 softmax scale into its scale operand.
"""

import numpy as np

P = 128
C = 512
KC = C // P          # 4 channel chunks of 128
N = 4096             # tokens (64*64)
NH = N // 2          # query tokens per core
G = 32               # groupnorm groups
GS = C // G          # 16 channels per group
EPS = 1e-6
N_CORES = 8

SCW = np.float32(16.0)       # fp8 pre-scale on weights/activations
MSH = 3.0                    # exp shift: exp(s - MSH), cancels in softmax
ESC = float(C ** -0.5 / (SCW * SCW))  # exp scale on raw fp8 score psum

NKB = N // P         # 32 key blocks of 128
NQC = NH // 512      # 4 query chunks of 512

_CACHE = {}


def _apply_walrus_workarounds():
    """The walrus build in this container rejects any instruction carrying
    more than one semaphore wait ("Too many sync wait commands"). Split extra
    waits onto same-engine single-wait NOPs committed just before, and split
    the final TileContext drain the same way."""
    import concourse.tile as tile
    from concourse import mybir

    if getattr(tile.TileContext, "_walrus_wait_split", False):
        return

    _orig_commit = tile.TileContext._commit_instruction

    def _split_waits_commit(self, inst, lazy_reg_writes=True):
        si = inst.sync_info
        if si is not None and si.on_wait and len(si.on_wait) > 1 \
                and inst.engine != mybir.EngineType.Unassigned:
            waits = list(si.on_wait)
            si.on_wait = waits[-1:]
            for w in waits[:-1]:
                nop = mybir.InstNoOp(
                    name=self.nc.get_next_instruction_name(),
                    engine=inst.engine,
                    sync_info=mybir.SyncInfo(on_wait=[w], on_update=[]),
                    bass_nofuse=True,
                )
                _orig_commit(self, nop, lazy_reg_writes=False)
        return _orig_commit(self, inst, lazy_reg_writes=lazy_reg_writes)

    def _split_drain_and_barrier(self, tick_clock, wait_clock):
        nc = self.nc
        drain_inst = nc.sync.drain()
        wait_clock.add_sem_waits(
            drain_inst.ins, tile.ScopedClock({None: tick_clock.global_clock})
        )
        si = drain_inst.ins.sync_info
        waits = list(si.on_wait) if si is not None else []
        if len(waits) > 1:
            si.on_wait = waits[:1]
            for w in waits[1:]:
                d2 = nc.sync.drain()
                d2.ins.sync_info = mybir.SyncInfo(on_wait=[w], on_update=[])

        import os
        nc.all_engine_barrier()
        assert self.sems is not None
        popped = nc._tile_sem_poison_stack.pop()
        assert popped is self._sem_poison
        if os.environ.get("KERNEL_SKIP_SEM_RESET") != "1":
            nc.clear_and_free_semaphores(list(self.sems.allocated().values()))
            nc.all_engine_barrier()

    tile.TileContext._commit_instruction = _split_waits_commit
    tile.TileContext._drain_and_barrier = _split_drain_and_barrier
    tile.TileContext._walrus_wait_split = True


def _build():
    """Trace the Bass/Tile program once; returns the Bass module."""
    import concourse.bass as bass
    import concourse.tile as tile
    from concourse import mybir

    _apply_walrus_workarounds()

    DT = mybir.dt.float16
    F8 = mybir.dt.float8e4
    F32 = mybir.dt.float32
    DR = mybir.MatmulPerfMode.DoubleRow

    nc = bass.Bass("TRN2", target_bir_lowering=False, debug=False, num_devices=1)

    xr = nc.dram_tensor("xr", [C, N], DT, kind="ExternalInput").ap()
    wq = nc.dram_tensor("wq", [C, C], F8, kind="ExternalInput").ap()
    wk = nc.dram_tensor("wk", [C, C], F8, kind="ExternalInput").ap()
    wv = nc.dram_tensor("wv", [C, C], F8, kind="ExternalInput").ap()
    # packed per-channel vectors: [16*bq, gamma, beta, wo@bv+bo]
    bvec = nc.dram_tensor("bvec", [4, C], F32, kind="ExternalInput").ap()
    gavg = nc.dram_tensor("gavg", [P, P], F32, kind="ExternalInput").ap()
    ident = nc.dram_tensor("ident", [P, P], DT, kind="ExternalInput").ap()
    y = nc.dram_tensor("y", [C, NH], F32, kind="ExternalOutput").ap()

    xr_t = xr.rearrange("(kc p) n -> kc p n", p=P)     # [4, 128, 4096]
    y_r = y.rearrange("(oc p) n -> p oc n", p=P)       # [128, 4, 2048]

    with tile.TileContext(nc) as tc:
        import contextlib
        ctx = contextlib.ExitStack()
        with ctx:
            consts = ctx.enter_context(tc.tile_pool(name="consts", bufs=1))
            big = ctx.enter_context(tc.tile_pool(name="big", bufs=1))
            small = ctx.enter_context(tc.tile_pool(name="small", bufs=4))
            epool = ctx.enter_context(tc.tile_pool(name="epool", bufs=4))
            rpool = ctx.enter_context(tc.tile_pool(name="rpool", bufs=3))
            ps = ctx.enter_context(tc.tile_pool(name="ps", bufs=8, space="PSUM"))

            # ---- consts (gpsimd SWDGE queue; ident first for PE warmup) ---
            ident_sb = consts.tile([P, P], DT, tag="ident")
            nc.gpsimd.dma_start(ident_sb[:], ident)
            warm_sb = consts.tile([P, 512], DT, tag="warm")
            nc.vector.memset(warm_sb[:], 0.5)
            ones8 = consts.tile([P, 2, P], F8, tag="ones")
            nc.vector.memset(ones8[:], float(SCW))
            expb = consts.tile([P, 1], F32, tag="expb")
            nc.vector.memset(expb[:], -MSH)
            eps_sb = consts.tile([P, 1], F32, tag="eps")
            nc.vector.memset(eps_sb[:], EPS)

            # PE clock warm-up: accumulation chain gated only on ident/warm
            # keeps the PE streaming from ~t=2us so HAM lifts the clock to
            # 2.4GHz while GroupNorm (DVE/ACT-bound) is still running. More
            # bursts are interleaved between the GroupNorm chunks below so
            # the PE never idles long enough for HAM to ramp back down.
            warm_ps = ps.tile([P, 512], F32, tag="mm", name="warm")
            NWARM = 48 + 4 * 24
            _warm_i = [0]

            def warm_burst(n):
                for _ in range(n):
                    wi = _warm_i[0]
                    _warm_i[0] += 1
                    nc.tensor.matmul(warm_ps[:, :256], ident_sb[:],
                                     warm_sb[:, :256],
                                     start=(wi == 0), stop=(wi == NWARM - 1))

            warm_burst(48)

            bv_sb = consts.tile([P, 4, KC], F32, tag="bvec")
            nc.gpsimd.dma_start(
                bv_sb[:], bvec.rearrange("v (kc p) -> p v kc", p=P))
            b_sb = {n: bv_sb[:, vi, :] for vi, n in
                    enumerate(("bq", "gam", "bet", "bo"))}
            gavg_sb = consts.tile([P, P], F32, tag="gavg")
            nc.gpsimd.dma_start(gavg_sb[:], gavg)

            # ---- phase 1: GroupNorm -> hn8 (fp8) --------------------------
            # x stays fully resident in SBUF (also serves the residual).
            hn8 = big.tile([P, KC, N], F8, tag="hn")
            x_full = big.tile([P, KC, N], DT, tag="xf")
            for kc in range(KC):
                x_c = x_full[:, kc, :]
                nc.sync.dma_start(x_c[:], xr_t[kc])
                # raw per-partition sum (DVE) and sum of squares (ScalarE
                # Square with fused accumulator; hn8[:, kc] is throwaway
                # scratch). The 1/(GS*N) normalization is folded into gavg.
                mv2 = small.tile([P, 2], F32, tag="mv2")
                nc.vector.tensor_reduce(
                    mv2[:, 0:1], x_c[:], mybir.AxisListType.X,
                    mybir.AluOpType.add)
                nc.scalar.activation(
                    hn8[:, kc, :], x_c[:], mybir.ActivationFunctionType.Square,
                    accum_out=mv2[:, 1:2])
                # group-average (and broadcast back to partitions) via PE
                g_ps = ps.tile([P, 2], F32, tag="mm", name=f"gn{kc}")
                nc.tensor.matmul(g_ps[:], gavg_sb[:], mv2[:], start=True, stop=True)

                # var_g = E2_g - mean_g^2 ; rstd = 1/sqrt(var_g + eps)
                g_sb = small.tile([P, 2], F32, tag="gsb")
                nc.vector.tensor_copy(g_sb[:], g_ps[:])
                var_t = small.tile([P, 1], F32, tag="var")
                nc.gpsimd.tensor_tensor(
                    var_t[:], g_sb[:, 0:1], g_sb[:, 0:1], mybir.AluOpType.mult)
                nc.gpsimd.tensor_tensor(
                    var_t[:], g_sb[:, 1:2], var_t[:], mybir.AluOpType.subtract)
                sq = small.tile([P, 1], F32, tag="sq")
                nc.scalar.activation(
                    sq[:], var_t[:], mybir.ActivationFunctionType.Sqrt,
                    bias=eps_sb[:], scale=1.0)
                rstd = small.tile([P, 1], F32, tag="rstd")
                nc.vector.reciprocal(rstd[:], sq[:])

                # scale = rstd * gamma ; shift = beta - mean_g * scale
                scl = small.tile([P, 1], F32, tag="scl")
                nc.gpsimd.tensor_tensor(
                    scl[:], rstd[:], b_sb["gam"][:, kc:kc + 1], mybir.AluOpType.mult)
                sh = small.tile([P, 1], F32, tag="sh")
                nc.gpsimd.tensor_tensor(
                    sh[:], g_sb[:, 0:1], scl[:], mybir.AluOpType.mult)
                nc.gpsimd.tensor_tensor(
                    sh[:], b_sb["bet"][:, kc:kc + 1], sh[:], mybir.AluOpType.subtract)

                nc.vector.tensor_scalar(
                    out=hn8[:, kc, :], in0=x_c[:], scalar1=scl[:], scalar2=sh[:],
                    op0=mybir.AluOpType.mult, op1=mybir.AluOpType.add)
                warm_burst(24)

            # weights (first needed by phase 2)
            w_sb = {}
            for name, ap in (("wk", wk), ("wq", wq), ("wv", wv)):
                t = consts.tile([P, KC, C], F8, tag=f"w_{name}")
                nc.gpsimd.dma_start(t[:], ap.rearrange("(kc p) o -> p kc o", p=P))
                w_sb[name] = t

            # ---- phase 2: projections (fp8 DoubleRow) --------------------
            k8 = big.tile([P, KC, N], F8, tag="k")
            q8 = big.tile([P, KC, NH], F8, tag="q")
            vt8 = big.tile([P, NKB, C], F8, tag="vt")

            for nt in range(N // 512):
                for oc in range(KC):
                    pp = ps.tile([P, 512], F32, tag="mm")
                    for kcp in range(2):
                        nc.tensor.matmul(
                            pp[:],
                            w_sb["wk"][:, 2 * kcp:2 * kcp + 2, oc * P:(oc + 1) * P],
                            hn8[:, 2 * kcp:2 * kcp + 2, nt * 512:(nt + 1) * 512],
                            start=(kcp == 0), stop=(kcp == 1), perf_mode=DR)
                    # bk cancels in softmax: pure cast epilogue, alternating
                    # DVE / ACT so neither engine gates the PE's psum banks
                    if (nt * KC + oc) % 2 == 0:
                        nc.vector.tensor_copy(
                            k8[:, oc, nt * 512:(nt + 1) * 512], pp[:])
                    else:
                        nc.scalar.activation(
                            k8[:, oc, nt * 512:(nt + 1) * 512], pp[:],
                            mybir.ActivationFunctionType.Copy, scale=1.0)
            for nt in range(NQC):
                for oc in range(KC):
                    pp = ps.tile([P, 512], F32, tag="mm")
                    for kcp in range(2):
                        nc.tensor.matmul(
                            pp[:],
                            w_sb["wq"][:, 2 * kcp:2 * kcp + 2, oc * P:(oc + 1) * P],
                            hn8[:, 2 * kcp:2 * kcp + 2, nt * 512:(nt + 1) * 512],
                            start=(kcp == 0), stop=(kcp == 1), perf_mode=DR)
                    nc.scalar.activation(
                        q8[:, oc, nt * 512:(nt + 1) * 512], pp[:],
                        mybir.ActivationFunctionType.Identity,
                        bias=b_sb["bq"][:, oc:oc + 1], scale=1.0)
            for jc in range(NKB):
                pp = ps.tile([P, 512], F32, tag="mm")
                for kcp in range(2):
                    nc.tensor.matmul(
                        pp[:],
                        hn8[:, 2 * kcp:2 * kcp + 2, jc * P:(jc + 1) * P],
                        w_sb["wv"][:, 2 * kcp:2 * kcp + 2, :],
                        start=(kcp == 0), stop=(kcp == 1), perf_mode=DR)
                if jc % 2 == 0:
                    nc.vector.tensor_copy(vt8[:, jc, :], pp[:])
                else:
                    nc.scalar.activation(
                        vt8[:, jc, :], pp[:],
                        mybir.ActivationFunctionType.Copy, scale=1.0)

            # ---- phase 3: attention over transposed scores ---------------
            for qc in range(NQC):
                qsl = slice(qc * 512, (qc + 1) * 512)
                den_ps = ps.tile([P, KC, P], F32, tag="mm", name=f"den{qc}")
                ot_ps = [ps.tile([P, C], F32, tag="mm", name=f"ot{qc}_{qb}")
                         for qb in range(4)]
                at_hold = {}

                def stage_s(kb, qc=qc, qsl=qsl, at_hold=at_hold):
                    p = kb // 2
                    if kb % 2 == 0:
                        at_hold[p] = epool.tile([P, 2, 512], F8, tag="at",
                                                name=f"at{qc}_{p}")
                    s_ps = ps.tile([P, 512], F32, tag="mm")
                    for kcp in range(2):
                        nc.tensor.matmul(
                            s_ps[:],
                            k8[:, 2 * kcp:2 * kcp + 2, kb * P:(kb + 1) * P],
                            q8[:, 2 * kcp:2 * kcp + 2, qsl],
                            start=(kcp == 0), stop=(kcp == 1), perf_mode=DR)
                    nc.scalar.activation(
                        at_hold[p][:, kb % 2, :], s_ps[:],
                        mybir.ActivationFunctionType.Exp,
                        bias=expb[:], scale=ESC)

                def stage_ot(p, den_ps=den_ps, ot_ps=ot_ps, at_hold=at_hold):
                    at = at_hold.pop(p)
                    for qb in range(4):
                        nc.tensor.matmul(
                            ot_ps[qb][:], at[:, :, qb * P:(qb + 1) * P],
                            vt8[:, 2 * p:2 * p + 2, :],
                            start=(p == 0), stop=(p == NKB // 2 - 1),
                            perf_mode=DR)
                        nc.tensor.matmul(
                            den_ps[:, qb, :], at[:, :, qb * P:(qb + 1) * P],
                            ones8[:],
                            start=(p == 0), stop=(p == NKB // 2 - 1),
                            perf_mode=DR)

                for kb in range(NKB):
                    stage_s(kb)
                    if kb % 2 == 1 and kb >= 3:
                        stage_ot((kb - 3) // 2)
                stage_ot(NKB // 2 - 1)

                # epilogue: scale by 1/den, transpose back to [c, q], add
                # residual + folded bias, DMA out. Two halves of 256 queries.
                tp = {}
                for qb in range(4):
                    recip = small.tile([P, 1], F32, tag="recip")
                    nc.vector.reciprocal(recip[:], den_ps[:, qb, 0:1])
                    ot_sb = rpool.tile([P, C], DT, tag="ot")
                    nc.scalar.activation(
                        ot_sb[:], ot_ps[qb][:],
                        mybir.ActivationFunctionType.Copy, scale=recip[:])
                    j, jj = divmod(qb, 2)
                    if jj == 0:
                        tp[j] = ps.tile([P, KC, 256], DT, tag="mm",
                                        name=f"tp{qc}_{j}")
                    for cb in range(KC):
                        nc.tensor.transpose(
                            tp[j][:, cb, jj * P:(jj + 1) * P],
                            ot_sb[:, cb * P:(cb + 1) * P], ident_sb[:])
                for j in range(2):
                    r1 = rpool.tile([P, KC, 256], F32, tag="r1")
                    rsl = slice(qc * 512 + j * 256, qc * 512 + (j + 1) * 256)
                    for cb in range(KC):
                        nc.vector.tensor_scalar_add(
                            r1[:, cb, :], tp[j][:, cb, :],
                            b_sb["bo"][:, cb:cb + 1])
                        nc.gpsimd.tensor_tensor(
                            r1[:, cb, :], r1[:, cb, :], x_full[:, cb, rsl],
                            mybir.AluOpType.add)
                    nc.sync.dma_start(y_r[:, :, rsl], r1[:])

    return nc


def _prep_in_maps(inputs):
    import ml_dtypes
    f8 = ml_dtypes.float8_e4m3

    x = np.asarray(inputs["x"], np.float32).reshape(4, C, N)
    wq = np.asarray(inputs["wq"], np.float32)
    wk = np.asarray(inputs["wk"], np.float32)
    wv = np.asarray(inputs["wv"], np.float32)
    wo = np.asarray(inputs["wo"], np.float32)
    wvp = wo @ wv                     # fold output projection into v
    bvec = np.stack([
        np.asarray(inputs["bq"], np.float32) * SCW,
        np.asarray(inputs["gamma"], np.float32),
        np.asarray(inputs["beta"], np.float32),
        wo @ np.asarray(inputs["bv"], np.float32)
        + np.asarray(inputs["bo"], np.float32),
    ]).astype(np.float32)
    shared = {
        "wq": np.ascontiguousarray((wq * SCW).T).astype(f8),
        "wk": np.ascontiguousarray((wk * SCW).T).astype(f8),
        "wv": np.ascontiguousarray((wvp * SCW).T).astype(f8),
        "bvec": bvec,
        "gavg": (np.kron(np.eye(P // GS, dtype=np.float32),
                         np.ones((GS, GS), np.float32)) / (GS * N)),
        "ident": np.eye(P, dtype=np.float16),
    }
    in_maps = []
    for core in range(N_CORES):
        b, half = divmod(core, 2)
        xb = x[b]
        if half == 1:
            xrot = np.ascontiguousarray(
                np.concatenate([xb[:, NH:], xb[:, :NH]], axis=1))
        else:
            xrot = np.ascontiguousarray(xb)
        in_maps.append({"xr": xrot.astype(np.float16), **shared})
    return in_maps


def kernel_run(inputs, trace=False, trace_cores=None):
    """Run on all 8 cores; returns (full_output, BassKernelResults)."""
    from concourse.bass_utils import run_bass_kernel_spmd

    if "nc" not in _CACHE:
        _CACHE["nc"] = _build()
    nc = _CACHE["nc"]
    in_maps = _prep_in_maps(inputs)
    res = run_bass_kernel_spmd(
        nc, in_maps, core_ids=list(range(N_CORES)), trace=trace,
        trace_cores=trace_cores)
    out = np.empty((4, C, N), np.float32)
    for core in range(N_CORES):
        b, half = divmod(core, 2)
        out[b][:, half * NH:(half + 1) * NH] = res.results[core]["y"]
    return out.reshape(4, C, 64, 64), res


def kernel(**inputs):
    out, _ = kernel_run(inputs, trace=False)
    return out


# revision 16
# speedup vs baseline: 1.6996x; 1.0152x over previous
"""AttnBlock (GroupNorm + single-head 1x1-conv attention + residual) on 8
Trainium2 NeuronCores.

Sharding: data-parallel over batch (4) x sequence-parallel over query tokens
(2 halves of 4096). Each core receives its batch element with the spatial
columns rotated so that its 2048 query tokens are always columns 0:2047 —
attention is invariant to key order, so one shared NEFF serves all cores.

Math/layout tricks vs the fp16 baseline:
  * All big matmuls run in fp8(e4m3) DoubleRow mode (2x PE throughput):
    weights, hn, q, k, v, and the attention weights are fp8.
  * Scores are computed TRANSPOSED (S^T[key, query]) so exp() output lands
    directly in the [key, query] layout the attn@V matmul needs as lhsT —
    no PE transposes and no DVE copies of the 16M-element score matrix.
  * Softmax denominators ride on piggy-backed DoubleRow matmuls that reuse
    the attention tile as stationary weights against a constant ones-rhs.
  * The key bias bk drops exactly (adds a per-query constant to scores ->
    cancels in softmax); wo is folded into wv on the host (wv' = wo @ wv),
    eliminating the entire output-projection phase; wo@bv + bo rides the
    residual add (softmax weights sum to one).
  * Weights/activations are pre-scaled by 16 so fp8 values sit in the
    normal-number range; the exp() activation folds the compensating
    1/256 and the C^-0.5 softmax scale into its scale operand.
"""

import numpy as np

P = 128
C = 512
KC = C // P          # 4 channel chunks of 128
N = 4096             # tokens (64*64)
NH = N // 2          # query tokens per core
G = 32               # groupnorm groups
GS = C // G          # 16 channels per group
EPS = 1e-6
N_CORES = 8

SCW = np.float32(16.0)       # fp8 pre-scale on weights/activations
MSH = 3.0                    # exp shift: exp(s - MSH), cancels in softmax
ESC = float(C ** -0.5 / (SCW * SCW))  # exp scale on raw fp8 score psum

NKB = N // P         # 32 key blocks of 128
NQC = NH // 512      # 4 query chunks of 512

_CACHE = {}


def _apply_walrus_workarounds():
    """The walrus build in this container rejects any instruction carrying
    more than one semaphore wait ("Too many sync wait commands"). Split extra
    waits onto same-engine single-wait NOPs committed just before, and split
    the final TileContext drain the same way."""
    import concourse.tile as tile
    from concourse import mybir

    if getattr(tile.TileContext, "_walrus_wait_split", False):
        return

    _orig_commit = tile.TileContext._commit_instruction

    def _split_waits_commit(self, inst, lazy_reg_writes=True):
        si = inst.sync_info
        if si is not None and si.on_wait and len(si.on_wait) > 1 \
                and inst.engine != mybir.EngineType.Unassigned:
            waits = list(si.on_wait)
            si.on_wait = waits[-1:]
            for w in waits[:-1]:
                nop = mybir.InstNoOp(
                    name=self.nc.get_next_instruction_name(),
                    engine=inst.engine,
                    sync_info=mybir.SyncInfo(on_wait=[w], on_update=[]),
                    bass_nofuse=True,
                )
                _orig_commit(self, nop, lazy_reg_writes=False)
        return _orig_commit(self, inst, lazy_reg_writes=lazy_reg_writes)

    def _split_drain_and_barrier(self, tick_clock, wait_clock):
        nc = self.nc
        drain_inst = nc.sync.drain()
        wait_clock.add_sem_waits(
            drain_inst.ins, tile.ScopedClock({None: tick_clock.global_clock})
        )
        si = drain_inst.ins.sync_info
        waits = list(si.on_wait) if si is not None else []
        if len(waits) > 1:
            si.on_wait = waits[:1]
            for w in waits[1:]:
                d2 = nc.sync.drain()
                d2.ins.sync_info = mybir.SyncInfo(on_wait=[w], on_update=[])

        import os
        nc.all_engine_barrier()
        assert self.sems is not None
        popped = nc._tile_sem_poison_stack.pop()
        assert popped is self._sem_poison
        if os.environ.get("KERNEL_SKIP_SEM_RESET") != "1":
            nc.clear_and_free_semaphores(list(self.sems.allocated().values()))
            nc.all_engine_barrier()

    tile.TileContext._commit_instruction = _split_waits_commit
    tile.TileContext._drain_and_barrier = _split_drain_and_barrier
    tile.TileContext._walrus_wait_split = True


def _build():
    """Trace the Bass/Tile program once; returns the Bass module."""
    import concourse.bass as bass
    import concourse.tile as tile
    from concourse import mybir

    _apply_walrus_workarounds()

    DT = mybir.dt.float16
    F8 = mybir.dt.float8e4
    F32 = mybir.dt.float32
    DR = mybir.MatmulPerfMode.DoubleRow

    nc = bass.Bass("TRN2", target_bir_lowering=False, debug=False, num_devices=1)

    xr = nc.dram_tensor("xr", [C, N], DT, kind="ExternalInput").ap()
    wq = nc.dram_tensor("wq", [C, C], F8, kind="ExternalInput").ap()
    wk = nc.dram_tensor("wk", [C, C], F8, kind="ExternalInput").ap()
    wv = nc.dram_tensor("wv", [C, C], F8, kind="ExternalInput").ap()
    # packed per-channel vectors: [16*bq, gamma, beta, wo@bv+bo]
    bvec = nc.dram_tensor("bvec", [4, C], F32, kind="ExternalInput").ap()
    gavg = nc.dram_tensor("gavg", [P, P], F32, kind="ExternalInput").ap()
    ident = nc.dram_tensor("ident", [P, P], DT, kind="ExternalInput").ap()
    y = nc.dram_tensor("y", [C, NH], F32, kind="ExternalOutput").ap()

    xr_t = xr.rearrange("(kc p) n -> kc p n", p=P)     # [4, 128, 4096]
    y_r = y.rearrange("(oc p) n -> p oc n", p=P)       # [128, 4, 2048]

    with tile.TileContext(nc) as tc:
        import contextlib
        ctx = contextlib.ExitStack()
        with ctx:
            consts = ctx.enter_context(tc.tile_pool(name="consts", bufs=1))
            big = ctx.enter_context(tc.tile_pool(name="big", bufs=1))
            small = ctx.enter_context(tc.tile_pool(name="small", bufs=4))
            epool = ctx.enter_context(tc.tile_pool(name="epool", bufs=4))
            rpool = ctx.enter_context(tc.tile_pool(name="rpool", bufs=3))
            ps = ctx.enter_context(tc.tile_pool(name="ps", bufs=8, space="PSUM"))

            # ---- consts (gpsimd SWDGE queue; ident first for PE warmup) ---
            ident_sb = consts.tile([P, P], DT, tag="ident")
            nc.gpsimd.dma_start(ident_sb[:], ident)
            warm_sb = consts.tile([P, 512], DT, tag="warm")
            nc.vector.memset(warm_sb[:], 0.5)
            ones8 = consts.tile([P, 2, P], F8, tag="ones")
            nc.vector.memset(ones8[:], float(SCW))
            expb = consts.tile([P, 1], F32, tag="expb")
            nc.vector.memset(expb[:], -MSH)
            eps_sb = consts.tile([P, 1], F32, tag="eps")
            nc.vector.memset(eps_sb[:], EPS)

            # PE clock warm-up: accumulation chain gated only on ident/warm
            # keeps the PE streaming from ~t=2us so HAM lifts the clock to
            # 2.4GHz while GroupNorm (DVE/ACT-bound) is still running. More
            # bursts are interleaved between the GroupNorm chunks below so
            # the PE never idles long enough for HAM to ramp back down.
            warm_ps = ps.tile([P, 512], F32, tag="mm", name="warm")
            NWARM = 64 + 4 * 24 + 32
            _warm_i = [0]

            def warm_burst(n):
                for _ in range(n):
                    wi = _warm_i[0]
                    _warm_i[0] += 1
                    nc.tensor.matmul(warm_ps[:, :256], ident_sb[:],
                                     warm_sb[:, :256],
                                     start=(wi == 0), stop=(wi == NWARM - 1))

            warm_burst(64)

            bv_sb = consts.tile([P, 4, KC], F32, tag="bvec")
            nc.gpsimd.dma_start(
                bv_sb[:], bvec.rearrange("v (kc p) -> p v kc", p=P))
            b_sb = {n: bv_sb[:, vi, :] for vi, n in
                    enumerate(("bq", "gam", "bet", "bo"))}
            gavg_sb = consts.tile([P, P], F32, tag="gavg")
            nc.gpsimd.dma_start(gavg_sb[:], gavg)

            # weights early on the SWDGE queue so phase 2 never waits
            w_sb = {}
            for name, ap in (("wk", wk), ("wq", wq), ("wv", wv)):
                t = consts.tile([P, KC, C], F8, tag=f"w_{name}")
                nc.gpsimd.dma_start(t[:], ap.rearrange("(kc p) o -> p kc o", p=P))
                w_sb[name] = t

            # ---- phase 1: GroupNorm -> hn8 (fp8) --------------------------
            # x stays fully resident in SBUF (also serves the residual).
            hn8 = big.tile([P, KC, N], F8, tag="hn")
            x_full = big.tile([P, KC, N], DT, tag="xf")
            for kc in range(KC):
                x_c = x_full[:, kc, :]
                nc.sync.dma_start(x_c[:], xr_t[kc])
                # raw per-partition sum (DVE) and sum of squares (ScalarE
                # Square with fused accumulator; hn8[:, kc] is throwaway
                # scratch). The 1/(GS*N) normalization is folded into gavg.
                mv2 = small.tile([P, 2], F32, tag="mv2")
                nc.vector.tensor_reduce(
                    mv2[:, 0:1], x_c[:], mybir.AxisListType.X,
                    mybir.AluOpType.add)
                nc.scalar.activation(
                    hn8[:, kc, :], x_c[:], mybir.ActivationFunctionType.Square,
                    accum_out=mv2[:, 1:2])
                # group-average (and broadcast back to partitions) via PE
                g_ps = ps.tile([P, 2], F32, tag="mm", name=f"gn{kc}")
                nc.tensor.matmul(g_ps[:], gavg_sb[:], mv2[:], start=True, stop=True)

                # var_g = E2_g - mean_g^2 ; rstd = 1/sqrt(var_g + eps)
                g_sb = small.tile([P, 2], F32, tag="gsb")
                nc.vector.tensor_copy(g_sb[:], g_ps[:])
                var_t = small.tile([P, 1], F32, tag="var")
                nc.gpsimd.tensor_tensor(
                    var_t[:], g_sb[:, 0:1], g_sb[:, 0:1], mybir.AluOpType.mult)
                nc.gpsimd.tensor_tensor(
                    var_t[:], g_sb[:, 1:2], var_t[:], mybir.AluOpType.subtract)
                sq = small.tile([P, 1], F32, tag="sq")
                nc.scalar.activation(
                    sq[:], var_t[:], mybir.ActivationFunctionType.Sqrt,
                    bias=eps_sb[:], scale=1.0)
                rstd = small.tile([P, 1], F32, tag="rstd")
                nc.vector.reciprocal(rstd[:], sq[:])

                # scale = rstd * gamma ; shift = beta - mean_g * scale
                scl = small.tile([P, 1], F32, tag="scl")
                nc.gpsimd.tensor_tensor(
                    scl[:], rstd[:], b_sb["gam"][:, kc:kc + 1], mybir.AluOpType.mult)
                sh = small.tile([P, 1], F32, tag="sh")
                nc.gpsimd.tensor_tensor(
                    sh[:], g_sb[:, 0:1], scl[:], mybir.AluOpType.mult)
                nc.gpsimd.tensor_tensor(
                    sh[:], b_sb["bet"][:, kc:kc + 1], sh[:], mybir.AluOpType.subtract)

                nc.vector.tensor_scalar(
                    out=hn8[:, kc, :], in0=x_c[:], scalar1=scl[:], scalar2=sh[:],
                    op0=mybir.AluOpType.mult, op1=mybir.AluOpType.add)
                warm_burst(24)

            warm_burst(32)

            # ---- phase 2: projections (fp8 DoubleRow) --------------------
            k8 = big.tile([P, KC, N], F8, tag="k")
            q8 = big.tile([P, KC, NH], F8, tag="q")
            vt8 = big.tile([P, NKB, C], F8, tag="vt")

            for nt in range(N // 512):
                for oc in range(KC):
                    pp = ps.tile([P, 512], F32, tag="mm")
                    for kcp in range(2):
                        nc.tensor.matmul(
                            pp[:],
                            w_sb["wk"][:, 2 * kcp:2 * kcp + 2, oc * P:(oc + 1) * P],
                            hn8[:, 2 * kcp:2 * kcp + 2, nt * 512:(nt + 1) * 512],
                            start=(kcp == 0), stop=(kcp == 1), perf_mode=DR)
                    # bk cancels in softmax: pure cast epilogue, alternating
                    # DVE / ACT so neither engine gates the PE's psum banks
                    if (nt * KC + oc) % 2 == 0:
                        nc.vector.tensor_copy(
                            k8[:, oc, nt * 512:(nt + 1) * 512], pp[:])
                    else:
                        nc.scalar.activation(
                            k8[:, oc, nt * 512:(nt + 1) * 512], pp[:],
                            mybir.ActivationFunctionType.Copy, scale=1.0)
            for nt in range(NQC):
                for oc in range(KC):
                    pp = ps.tile([P, 512], F32, tag="mm")
                    for kcp in range(2):
                        nc.tensor.matmul(
                            pp[:],
                            w_sb["wq"][:, 2 * kcp:2 * kcp + 2, oc * P:(oc + 1) * P],
                            hn8[:, 2 * kcp:2 * kcp + 2, nt * 512:(nt + 1) * 512],
                            start=(kcp == 0), stop=(kcp == 1), perf_mode=DR)
                    nc.scalar.activation(
                        q8[:, oc, nt * 512:(nt + 1) * 512], pp[:],
                        mybir.ActivationFunctionType.Identity,
                        bias=b_sb["bq"][:, oc:oc + 1], scale=1.0)
            for jc in range(NKB):
                pp = ps.tile([P, 512], F32, tag="mm")
                for kcp in range(2):
                    nc.tensor.matmul(
                        pp[:],
                        hn8[:, 2 * kcp:2 * kcp + 2, jc * P:(jc + 1) * P],
                        w_sb["wv"][:, 2 * kcp:2 * kcp + 2, :],
                        start=(kcp == 0), stop=(kcp == 1), perf_mode=DR)
                if jc % 2 == 0:
                    nc.vector.tensor_copy(vt8[:, jc, :], pp[:])
                else:
                    nc.scalar.activation(
                        vt8[:, jc, :], pp[:],
                        mybir.ActivationFunctionType.Copy, scale=1.0)

            # ---- phase 3: attention over transposed scores ---------------
            for qc in range(NQC):
                qsl = slice(qc * 512, (qc + 1) * 512)
                den_ps = ps.tile([P, KC, P], F32, tag="mm", name=f"den{qc}")
                ot_ps = [ps.tile([P, C], F32, tag="mm", name=f"ot{qc}_{qb}")
                         for qb in range(4)]
                at_hold = {}

                def stage_s(kb, qc=qc, qsl=qsl, at_hold=at_hold):
                    p = kb // 2
                    if kb % 2 == 0:
                        at_hold[p] = epool.tile([P, 2, 512], F8, tag="at",
                                                name=f"at{qc}_{p}")
                    s_ps = ps.tile([P, 512], F32, tag="mm")
                    for kcp in range(2):
                        nc.tensor.matmul(
                            s_ps[:],
                            k8[:, 2 * kcp:2 * kcp + 2, kb * P:(kb + 1) * P],
                            q8[:, 2 * kcp:2 * kcp + 2, qsl],
                            start=(kcp == 0), stop=(kcp == 1), perf_mode=DR)
                    nc.scalar.activation(
                        at_hold[p][:, kb % 2, :], s_ps[:],
                        mybir.ActivationFunctionType.Exp,
                        bias=expb[:], scale=ESC)

                def stage_ot(p, den_ps=den_ps, ot_ps=ot_ps, at_hold=at_hold):
                    at = at_hold.pop(p)
                    for qb in range(4):
                        nc.tensor.matmul(
                            ot_ps[qb][:], at[:, :, qb * P:(qb + 1) * P],
                            vt8[:, 2 * p:2 * p + 2, :],
                            start=(p == 0), stop=(p == NKB // 2 - 1),
                            perf_mode=DR)
                        nc.tensor.matmul(
                            den_ps[:, qb, :], at[:, :, qb * P:(qb + 1) * P],
                            ones8[:],
                            start=(p == 0), stop=(p == NKB // 2 - 1),
                            perf_mode=DR)

                for kb in range(NKB):
                    stage_s(kb)
                    if kb % 2 == 1 and kb >= 3:
                        stage_ot((kb - 3) // 2)

                # last pair fused with the epilogue so the scale/transpose/
                # residual chain of qb starts while qb+1..3 still matmul:
                # scale by 1/den, transpose back to [c, q], add residual +
                # folded bias, DMA out. Two halves of 256 queries.
                pl = NKB // 2 - 1
                at = at_hold.pop(pl)
                tp = {}
                for qb in range(4):
                    nc.tensor.matmul(
                        ot_ps[qb][:], at[:, :, qb * P:(qb + 1) * P],
                        vt8[:, 2 * pl:2 * pl + 2, :],
                        start=False, stop=True, perf_mode=DR)
                    nc.tensor.matmul(
                        den_ps[:, qb, :], at[:, :, qb * P:(qb + 1) * P],
                        ones8[:], start=False, stop=True, perf_mode=DR)
                    recip = small.tile([P, 1], F32, tag="recip")
                    nc.vector.reciprocal(recip[:], den_ps[:, qb, 0:1])
                    ot_sb = rpool.tile([P, C], DT, tag="ot")
                    nc.scalar.activation(
                        ot_sb[:], ot_ps[qb][:],
                        mybir.ActivationFunctionType.Copy, scale=recip[:])
                    j, jj = divmod(qb, 2)
                    if jj == 0:
                        tp[j] = ps.tile([P, KC, 256], DT, tag="mm",
                                        name=f"tp{qc}_{j}")
                    for cb in range(KC):
                        nc.tensor.transpose(
                            tp[j][:, cb, jj * P:(jj + 1) * P],
                            ot_sb[:, cb * P:(cb + 1) * P], ident_sb[:])
                    if jj == 1:
                        r1 = rpool.tile([P, KC, 256], F32, tag="r1")
                        rsl = slice(qc * 512 + j * 256,
                                    qc * 512 + (j + 1) * 256)
                        for cb in range(KC):
                            nc.vector.tensor_scalar_add(
                                r1[:, cb, :], tp[j][:, cb, :],
                                b_sb["bo"][:, cb:cb + 1])
                            nc.gpsimd.tensor_tensor(
                                r1[:, cb, :], r1[:, cb, :], x_full[:, cb, rsl],
                                mybir.AluOpType.add)
                        nc.sync.dma_start(y_r[:, :, rsl], r1[:])

    return nc


def _prep_in_maps(inputs):
    import ml_dtypes
    f8 = ml_dtypes.float8_e4m3

    x = np.asarray(inputs["x"], np.float32).reshape(4, C, N)
    wq = np.asarray(inputs["wq"], np.float32)
    wk = np.asarray(inputs["wk"], np.float32)
    wv = np.asarray(inputs["wv"], np.float32)
    wo = np.asarray(inputs["wo"], np.float32)
    wvp = wo @ wv                     # fold output projection into v
    bvec = np.stack([
        np.asarray(inputs["bq"], np.float32) * SCW,
        np.asarray(inputs["gamma"], np.float32),
        np.asarray(inputs["beta"], np.float32),
        wo @ np.asarray(inputs["bv"], np.float32)
        + np.asarray(inputs["bo"], np.float32),
    ]).astype(np.float32)
    shared = {
        "wq": np.ascontiguousarray((wq * SCW).T).astype(f8),
        "wk": np.ascontiguousarray((wk * SCW).T).astype(f8),
        "wv": np.ascontiguousarray((wvp * SCW).T).astype(f8),
        "bvec": bvec,
        "gavg": (np.kron(np.eye(P // GS, dtype=np.float32),
                         np.ones((GS, GS), np.float32)) / (GS * N)),
        "ident": np.eye(P, dtype=np.float16),
    }
    in_maps = []
    for core in range(N_CORES):
        b, half = divmod(core, 2)
        xb = x[b]
        if half == 1:
            xrot = np.ascontiguousarray(
                np.concatenate([xb[:, NH:], xb[:, :NH]], axis=1))
        else:
            xrot = np.ascontiguousarray(xb)
        in_maps.append({"xr": xrot.astype(np.float16), **shared})
    return in_maps


def kernel_run(inputs, trace=False, trace_cores=None):
    """Run on all 8 cores; returns (full_output, BassKernelResults)."""
    from concourse.bass_utils import run_bass_kernel_spmd

    if "nc" not in _CACHE:
        _CACHE["nc"] = _build()
    nc = _CACHE["nc"]
    in_maps = _prep_in_maps(inputs)
    res = run_bass_kernel_spmd(
        nc, in_maps, core_ids=list(range(N_CORES)), trace=trace,
        trace_cores=trace_cores)
    out = np.empty((4, C, N), np.float32)
    for core in range(N_CORES):
        b, half = divmod(core, 2)
        out[b][:, half * NH:(half + 1) * NH] = res.results[core]["y"]
    return out.reshape(4, C, 64, 64), res


def kernel(**inputs):
    out, _ = kernel_run(inputs, trace=False)
    return out
